# revision 59
# baseline (speedup 1.0000x reference)
"""Deformable conv block (3x3, offsets from a conv) on 8 TRN2 NeuronCores.

Self-contained: kernel(**inputs) takes full numpy inputs, shards
data-parallel over (batch, H-half) across 8 cores, runs one SPMD Bass
program per core via run_bass_kernel_spmd, and reassembles the full
output. All FLOPs (offset conv, bilinear sampling via GPSIMD
indirect_copy gather, main conv) run on device.

The kernel is bound by GPSIMD indirect_copy throughput (~27-34ns per
gathered index column, 1024-element/partition cap per instruction). The
input slab is host-packed as an interleaved "quad" layout of fp32 words
whose bf16 halves hold horizontal neighbor pairs: word 2j = (x[j],
x[j+1]), word 2j+1 = (x[j+133], x[j+134]). A single d=2 indirect_copy
column then fetches all FOUR bilinear corners of one sample, so one
instruction covers 512 samples/partition-group; the two partition-group
halves (0-3 / 4-7, slab duplicated) gather two different 512-pixel
halves of each 1024-pixel block. 72 gathers total instead of 144
one-corner-pair gathers. The offset conv reads the even-word lo-halves
of the same packed slab via a stride-4 bf16 AP with bf16 weights.

Corner weighting: per (block, tap, corner q) a one-hot matmul
broadcasts the transposed corner weights (wcmpB2 [72 = (h, t, q), 512])
to a [128, 512] PSUM tile (rows differ per pixel-half h), which
multiplies the strided bf16 corner view of the gathered tile; the main
conv then contracts 64 channels per pixel-half, accumulating 36
matmuls into each of two PSUM banks per block.

Sample order within a half (per tap, per 8-row block): s = m8h*128 +
r*16 + i with pixel (row hb+r, col 16*(4h+m8h)+i).
"""
import numpy as np
import ml_dtypes

import concourse.bass as bass
import concourse.mybir as mybir
import concourse.tile as tile_mod
from concourse import tile
from concourse.vector_clock import ScopedClock

# ---------------------------------------------------------------------------
# Patch 1: this container's walrus accepts at most ONE sync wait per
# instruction; split the tile-exit drain's waits across preceding SP nops.
def _drain_and_barrier(self, tick_clock, wait_clock):
    nc = self.nc
    carriers = [nc.sync.nop(nofuse=True, hint=f"drainwait{i}") for i in range(32)]
    drain_inst = nc.sync.drain()
    wait_clock.add_sem_waits(drain_inst.ins, ScopedClock({None: tick_clock.global_clock}))
    si = drain_inst.ins.sync_info
    waits = list(si.on_wait or [])
    if len(waits) > 1:
        si.on_wait = waits[:1]
        for i, w in enumerate(waits[1:]):
            ci = carriers[i].ins
            if ci.sync_info is None:
                ci.sync_info = mybir.SyncInfo(on_wait=[w], on_update=[])
            else:
                ci.sync_info.on_wait = (ci.sync_info.on_wait or []) + [w]
    nc.all_engine_barrier()
    assert self.sems is not None
    popped = nc._tile_sem_poison_stack.pop()
    assert popped is self._sem_poison
    nc.clear_and_free_semaphores(list(self.sems.allocated().values()))
    nc.all_engine_barrier()

tile_mod.TileContext._drain_and_barrier = _drain_and_barrier

# Patch 2: split multi-wait instructions everywhere (same walrus limit).
_ctr = [0]

def _mk_nop(engine, wait):
    _ctr[0] += 1
    nop = mybir.InstNoOp(name=f"WSPLIT-{_ctr[0]}", ins=[], outs=[])
    nop.engine = engine
    nop.sync_info = mybir.SyncInfo(on_wait=[wait], on_update=[])
    return nop

def split_waits(nc):
    n = 0
    for fn in nc.m.functions:
        for bb in fn.blocks:
            insts = list(bb.instructions)
            outl, changed = [], False
            for inst in insts:
                si = inst.sync_info
                if si is not None and si.on_wait and len(si.on_wait) > 1:
                    waits = list(si.on_wait)
                    for w in waits[:-1]:
                        nop = _mk_nop(inst.engine, w)
                        nc.register_instruction(nop, overwrite=True)
                        outl.append(nop)
                        n += 1
                    si.on_wait = waits[-1:]
                    inst.sync_info = si
                    changed = True
                outl.append(inst)
            if changed:
                bb.instructions = outl
    return n

# ---------------------------------------------------------------------------
F32 = mybir.dt.float32
F32R = mybir.dt.float32r
BF16 = mybir.dt.bfloat16
I32 = mybir.dt.int32
U16 = mybir.dt.uint16
AO = mybir.AluOpType
AP = bass.AP

B, Cin, Cout, H, W = 4, 64, 64, 128, 128
KK = 9
PADW = 133              # padded cols: x+2 for x in [-2, 130]
SLABROWS = 84           # slab rows: global-padded h0-8 .. h0+75
RBLK = 8
NBLK = 8
WIN_ROWS = 21
WIN = WIN_ROWS * PADW   # 3724
NS = RBLK * W           # 1024 samples per (block, tap)
NPIX = 64 * W
IDXF = KK * 64          # idxA free size per partition (576)
IWF2 = KK * NBLK * 32   # gather-idx free size per partition (2304)
SLABW = SLABROWS * PADW


def build_program():
    nc = bass.Bass()
    xpack = nc.declare_dram_parameter("xpack", [Cin, 2 * SLABW], F32, isOutput=False)
    xbf16 = nc.declare_dram_parameter("xbf16", [Cin, SLABW], BF16, isOutput=False)
    cl_lo = nc.declare_dram_parameter("cl_lo", [128, 576], F32, isOutput=False)
    cl_hi = nc.declare_dram_parameter("cl_hi", [128, 576], F32, isOutput=False)
    baseY = nc.declare_dram_parameter("baseY", [128, 576], F32, isOutput=False)
    baseX = nc.declare_dram_parameter("baseX", [128, 576], F32, isOutput=False)
    ident = nc.declare_dram_parameter("ident", [128, 128], F32, isOutput=False)
    offwT = nc.declare_dram_parameter("offwT", [Cin, KK * 18], BF16, isOutput=False)
    mainWT = nc.declare_dram_parameter("mainWT", [128, KK * Cout], F32R, isOutput=False)
    offb = nc.declare_dram_parameter("offb", [18, 1], F32, isOutput=False)
    mainb = nc.declare_dram_parameter("mainb", [Cout, 1], F32, isOutput=False)
    ohAll = nc.declare_dram_parameter("ohAll", [128, 36 * 128], F32R, isOutput=False)
    cbB = nc.declare_dram_parameter("cbB", [128, 576], F32, isOutput=False)
    out = nc.declare_dram_parameter("out", [Cout, NPIX], F32, isOutput=True)

    with tile.TileContext(nc) as tc:
        with (
            tc.tile_pool(name="big", bufs=1) as big,
            tc.tile_pool(name="ps", bufs=4, space="PSUM") as ps,
            tc.tile_pool(name="psacc", bufs=2, space="PSUM") as psa,
            tc.tile_pool(name="work", bufs=2) as wk,
        ):
            xcv = big.tile([128, SLABW], BF16, tag="xcv")
            nc.sync.dma_start(xcv[0:64, :], xbf16[:, :])
            xq = big.tile([128, 2 * SLABW], F32, tag="xq")
            woff = big.tile([Cin, KK * 18], BF16, tag="woff")
            nc.sync.dma_start(woff[:, :], offwT[:, :])
            wmain = big.tile([128, KK * Cout], F32R, tag="wmain")
            nc.scalar.dma_start(wmain[:, :], mainWT[:, :])
            bY = big.tile([128, 576], F32, tag="bY")
            nc.sync.dma_start(bY[:, :], baseY[:, :])
            bX = big.tile([128, 576], F32, tag="bX")
            nc.scalar.dma_start(bX[:, :], baseX[:, :])
            cLo = big.tile([128, 576], F32, tag="cLo")
            nc.sync.dma_start(cLo[:, :], cl_lo[:, :])
            cHi = big.tile([128, 576], F32, tag="cHi")
            nc.scalar.dma_start(cHi[:, :], cl_hi[:, :])
            idn = big.tile([128, 128], F32, tag="idn")
            nc.sync.dma_start(idn[:, :], ident[:, :])
            idnb = big.tile([128, 128], BF16, tag="idnb")
            nc.vector.tensor_copy(idnb[:, :], idn[:, :])
            ob = big.tile([18, 1], F32, tag="ob")
            nc.scalar.dma_start(ob[:, :], offb[:, :])
            mb = big.tile([Cout, 1], F32, tag="mb")
            nc.sync.dma_start(mb[:, :], mainb[:, :])
            oht = big.tile([128, 36 * 128], F32R, tag="oht")
            cb = big.tile([128, 576], F32, tag="cb")
            nc.sync.dma_start(cb[:, :], cbB[:, :])

            offT = big.tile([128, 64 * 18], F32, tag="offT")
            with tc.tile_pool(name="tmp", bufs=1) as tmp:
                # ---- 1. offset conv (bf16 matmuls off dedicated conv slab),
                # split into two tiles so the h<32 transposes overlap the
                # second half of the conv. ----
                offsA = tmp.tile([18, NPIX // 2], BF16, tag="offsA")
                offsB = tmp.tile([18, NPIX // 2], BF16, tag="offsB")
                offsAB = [offsA, offsB]
                for ch in range(16):
                    pt = ps.tile([18, 512], F32, tag="pp")
                    h0c = ch * 4
                    for t in range(KK):
                        ki, kj = t // 3, t % 3
                        off0 = (h0c + ki - 1 + 10) * PADW + (kj - 1 + 2)
                        rhs = AP(xcv[:].tensor, xcv[:].offset + off0,
                                 [[SLABW, 64], [PADW, 4], [1, 128]])
                        nc.tensor.matmul(pt[:, :],
                                         woff[:, t * 18:(t + 1) * 18],
                                         rhs,
                                         start=(t == 0), stop=(t == KK - 1))
                    dst = offsAB[ch // 8]
                    c8 = ch % 8
                    nc.vector.tensor_scalar(dst[:, c8 * 512:(c8 + 1) * 512], pt[:, :],
                                            ob[:, 0:1], None, AO.add)

                # ---- 2. transpose offsets -> offT [128w, (64h, 18)];
                # 4 rows per PSUM tile -> one 72-wide copy each. ----
                with tc.tile_pool(name="psT", bufs=2, space="PSUM") as psT:
                    for h4 in range(16):
                        offsh = offsAB[h4 // 8]
                        tp4 = psT.tile([128, 72], BF16, tag="pt4")
                        for j in range(4):
                            h32 = (h4 % 8) * 4 + j
                            nc.tensor.transpose(tp4[:, j * 18:(j + 1) * 18],
                                                offsh[:, h32 * 128:(h32 + 1) * 128],
                                                idnb[0:18, 0:18])
                        ov = AP(offT[:].tensor, offT[:].offset + h4 * 72,
                                [[64 * 18, 128], [1, 72]])
                        nc.vector.tensor_copy(ov, tp4[:, :])

            # quad slab + one-hot bank: only needed by the block phase, so
            # their DMAs overlap the conv/transpose work above.
            nc.scalar.dma_start(xq[0:64, :], xpack[:, :])
            nc.sync.dma_start(xq[64:128, :], xpack[:, :])
            nc.scalar.dma_start(oht[:, :], ohAll[:, :])

            # ---- 3. index/weight pipeline, all 9 taps batched [128, 576] ----
            # Free layout (t*64 + h); tap constants (ki-1, kj-1) folded into
            # the host tables bY/bX. d=2 gather consumes element-granular
            # indices: pair j lives at word 2j, so index math is 2-scaled;
            # cb[p][t*64+h] = 6 - (h//8)*8 folds the per-block window base.
            wcc = big.tile([128, KK * 4 * 64], F32, tag="wcc")
            idxA = big.tile([128, IDXF], U16, tag="idxA")
            with tc.tile_pool(name="wk2", bufs=1) as wk2:
                dyv = AP(offT[:].tensor, offT[:].offset + 0,
                         [[64 * 18, 128], [2, 9], [18, 64]])
                dxv = AP(offT[:].tensor, offT[:].offset + 1,
                         [[64 * 18, 128], [2, 9], [18, 64]])
                py = wk2.tile([128, 576], F32, tag="py")
                px = wk2.tile([128, 576], F32, tag="px")
                nc.vector.tensor_tensor(py[:, :], dyv, bY[:, :], AO.add)
                nc.vector.tensor_tensor(py[:, :], py[:, :], cLo[:, :], AO.max)
                nc.vector.tensor_tensor(py[:, :], py[:, :], cHi[:, :], AO.min)
                nc.vector.tensor_tensor(px[:, :], dxv, bX[:, :], AO.add)
                nc.vector.tensor_scalar(px[:, :], px[:, :], -2.0, 129.0, AO.max, AO.min)
                y0f = wk2.tile([128, 576], F32, tag="y0f")
                x0f = wk2.tile([128, 576], F32, tag="x0f")
                tmp2 = wk2.tile([128, 576], F32, tag="tmp2")
                y0i = wk2.tile([128, 576], I32, tag="i0")
                nc.vector.tensor_scalar(tmp2[:, :], py[:, :], 0.5, None, AO.subtract)
                nc.vector.tensor_copy(y0i[:, :], tmp2[:, :])
                nc.vector.tensor_copy(y0f[:, :], y0i[:, :])
                x0i = wk2.tile([128, 576], I32, tag="i0")
                nc.vector.tensor_scalar(tmp2[:, :], px[:, :], 0.5, None, AO.subtract)
                nc.vector.tensor_copy(x0i[:, :], tmp2[:, :])
                nc.vector.tensor_copy(x0f[:, :], x0i[:, :])
                ly = wk2.tile([128, 576], F32, tag="ly")
                lx = wk2.tile([128, 576], F32, tag="lx")
                my = wk2.tile([128, 576], F32, tag="my")
                mx = wk2.tile([128, 576], F32, tag="mx")
                nc.vector.tensor_tensor(ly[:, :], py[:, :], y0f[:, :], AO.subtract)
                nc.vector.tensor_tensor(lx[:, :], px[:, :], x0f[:, :], AO.subtract)
                nc.vector.tensor_scalar(my[:, :], ly[:, :], -1.0, 1.0, AO.mult, AO.add)
                nc.vector.tensor_scalar(mx[:, :], lx[:, :], -1.0, 1.0, AO.mult, AO.add)
                for r, (a, bb) in enumerate([(my, mx), (my, lx), (ly, mx), (ly, lx)]):
                    wv = AP(wcc[:].tensor, wcc[:].offset + r * 64,
                            [[KK * 4 * 64, 128], [256, 9], [1, 64]])
                    av = AP(a[:].tensor, a[:].offset, [[576, 128], [64, 9], [1, 64]])
                    bv = AP(bb[:].tensor, bb[:].offset, [[576, 128], [64, 9], [1, 64]])
                    nc.vector.tensor_tensor(wv, av, bv, AO.mult)
                nc.vector.tensor_scalar(x0f[:, :], x0f[:, :], 2.0, 4.0,
                                        AO.mult, AO.add)
                tb = wk2.tile([128, 576], F32, tag="tb")
                nc.vector.tensor_tensor(tb[:, :], y0f[:, :], cb[:, :], AO.add)
                nc.vector.tensor_scalar(tb[:, :], tb[:, :], 266.0, None, AO.mult)
                nc.vector.tensor_tensor(tb[:, :], tb[:, :], x0f[:, :], AO.add)
                nc.vector.tensor_scalar(tb[:, :], tb[:, :], 0.0,
                                        float(2 * (WIN - 135)), AO.max, AO.min)
                nc.vector.tensor_copy(idxA[:, :], tb[:, :])

            # ---- 4. build gather-idx tensor iw2 [128, (t, blk, m8h, r)] ----
            # iw2[16g+i][t*256 + blk*32 + m8h*8 + r] =
            #   idxA[16*(4h+m8h)+i][t*64+blk*8+r], h = g//4.
            iw2 = big.tile([128, IWF2], U16, tag="iw2")
            iwA1a = big.tile([16, 4 * IDXF], U16, tag="iwA1a")
            iwA1b = big.tile([16, 4 * IDXF], U16, tag="iwA1b")
            halves = [iwA1a, iwA1b]
            for m8 in range(8):
                eng = nc.sync if (m8 % 2 == 0) else nc.scalar
                dstt = halves[m8 // 4]
                m4 = m8 % 4
                eng.dma_start(dstt[0:16, m4 * IDXF:(m4 + 1) * IDXF],
                              idxA[16 * m8:16 * (m8 + 1), :])
            for t in range(KK):
                for hh, pbase in ((0, 0), (1, 64)):
                    srct = halves[hh]
                    src = AP(srct[:].tensor, srct[:].offset + t * 64,
                             [[4 * IDXF, 16], [IDXF, 4], [8, 8], [1, 8]])
                    dst = AP(iw2[:].tensor,
                             iw2[:].offset + pbase * IWF2 + t * 256,
                             [[IWF2, 16], [8, 4], [32, 8], [1, 8]])
                    nc.vector.tensor_copy(dst, src)
            for g in (1, 2, 3):
                eng = (nc.sync, nc.scalar, nc.sync)[g - 1]
                eng.dma_start(iw2[16 * g:16 * (g + 1), :], iw2[0:16, :])
            for g in (5, 6, 7):
                eng = (nc.scalar, nc.sync, nc.scalar)[g - 5]
                eng.dma_start(iw2[16 * g:16 * (g + 1), :], iw2[64:80, :])

            # ---- 5/6/7 per block ----
            with tc.tile_pool(name="g", bufs=3) as gp:
                for blk in range(NBLK):
                    hb = blk * RBLK
                    # corner weights: wcmpB2 [128 rows (h*64 + t*4 + q), 512 px]
                    # rows 36-63 / 100-127 are zeroed (one-hot lhsT is 0 there).
                    wcmpB2 = wk.tile([128, 512], F32R, tag="wcmpB2")
                    nc.vector.memset(wcmpB2[:, :].bitcast(F32), 0.0)
                    for hh in range(RBLK):
                        tp36 = ps.tile([36, 128], F32, tag="pp")
                        wsl36 = AP(wcc[:].tensor, wcc[:].offset + (hb + hh),
                                   [[KK * 4 * 64, 128], [64, KK * 4]])
                        nc.tensor.transpose(tp36[:, :], wsl36, idn[:, :])
                        for h in range(2):
                            dstw = AP(wcmpB2[:].tensor,
                                      wcmpB2[:].offset + h * 64 * 512 + hh * 16,
                                      [[512, 36], [128, 4], [1, 16]])
                            srcw = AP(tp36[:].tensor, tp36[:].offset + h * 64,
                                      [[128, 36], [16, 4], [1, 16]])
                            nc.vector.tensor_copy(dstw, srcw)
                    pt3a = psa.tile([Cout, 512], F32, tag="acc")
                    pt3b = psa.tile([Cout, 512], F32, tag="acc")
                    for t in range(KK):
                        gq = gp.tile([128, NS], F32, tag="gq")
                        iview = AP(iw2[:].tensor,
                                   iw2[:].offset + t * (NBLK * 32) + blk * 32,
                                   [[IWF2, 128], [1, 32]])
                        win0 = (hb + 4) * PADW
                        dataQ = AP(xq[:].tensor, xq[:].offset + 2 * win0,
                                   [[2 * SLABW, 128], [2, WIN - 1], [1, 2]])
                        nc.gpsimd.indirect_copy(
                            gq[:].rearrange("p (n i) -> p n i", i=2), dataQ,
                            iview, True)
                        gqb = gq[:].bitcast(BF16)
                        wsl2a = wmain[0:64, t * Cout:(t + 1) * Cout]
                        wsl2b = wmain[64:128, t * Cout:(t + 1) * Cout]
                        for q in range(4):
                            wq = ps.tile([128, 512], F32, tag="pp")
                            nc.tensor.matmul(
                                wq[:, :],
                                oht[:, (t * 4 + q) * 128:(t * 4 + q + 1) * 128],
                                wcmpB2[:, :],
                                start=True, stop=True)
                            vq = AP(gqb.tensor, gqb.offset + q,
                                    [[2 * NS, 128], [4, 512]])
                            gw = gp.tile([128, 512], F32R, tag="gw")
                            nc.vector.tensor_tensor(gw[:, :], vq, wq[:, :], AO.mult)
                            nc.tensor.matmul(pt3a[:, :], wsl2a, gw[0:64, :],
                                             start=(t == 0 and q == 0),
                                             stop=(t == KK - 1 and q == 3))
                            nc.tensor.matmul(pt3b[:, :], wsl2b, gw[64:128, :],
                                             start=(t == 0 and q == 0),
                                             stop=(t == KK - 1 and q == 3))
                    # bias add + un-permute s=(m8h, r, i) -> row-major, 1 DMA
                    ot = wk.tile([Cout, 1024], F32, tag="ot")
                    dstA = AP(ot[:].tensor, ot[:].offset,
                              [[1024, Cout], [16, 4], [128, 8], [1, 16]])
                    nc.vector.tensor_scalar(dstA, pt3a[:, :], mb[:, 0:1], None, AO.add)
                    dstB = AP(ot[:].tensor, ot[:].offset + 64,
                              [[1024, Cout], [16, 4], [128, 8], [1, 16]])
                    nc.vector.tensor_scalar(dstB, pt3b[:, :], mb[:, 0:1], None, AO.add)
                    eng = nc.sync if (blk % 2 == 0) else nc.scalar
                    eng.dma_start(out[:, blk * NS:(blk + 1) * NS], ot[:, :])
    return nc


def make_host_consts():
    """Input-independent constants shared by all cores."""
    c = {}
    # 576-wide (t*64 + h) tables with per-tap ki-1 / kj-1 folded in.
    hs = np.arange(64, dtype=np.float32)
    ki = (np.arange(KK) // 3).astype(np.float32) - 1.0
    kj = (np.arange(KK) % 3).astype(np.float32) - 1.0
    bY = (ki[:, None] + hs[None, :]).reshape(1, 576)
    c["baseY"] = np.tile(bY, (128, 1)).astype(np.float32)
    ps_ = np.arange(128, dtype=np.float32)[:, None, None]
    c["baseX"] = np.ascontiguousarray(
        (ps_ + kj[None, :, None] + np.zeros((1, 1, 64))).reshape(128, 576)
    ).astype(np.float32)
    c["ident"] = np.eye(128, dtype=np.float32)
    # one-hot lhsT bank: oh[h*64 + t4q, (t4q)*128 + h*64 + c] = 1.
    # For target (t, q): partition half h of the [128, 512] broadcast gets
    # wcmpB2 row h*64 + t*4 + q (that half's pixel set).
    oh = np.zeros((128, 36 * 128), np.float32)
    for t in range(KK):
        for q in range(4):
            t4q = t * 4 + q
            for h in range(2):
                oh[h * 64 + t4q, t4q * 128 + h * 64:t4q * 128 + (h + 1) * 64] = 1.0
    c["ohAll"] = oh
    cb1 = (6.0 - (np.arange(64) // 8) * 8).astype(np.float32)
    c["cbB"] = np.tile(np.tile(cb1, KK)[None, :], (128, 1))
    return c


def make_in_maps(x, offset_w, offset_b, weight, bias):
    consts = make_host_consts()
    offwT = np.ascontiguousarray(
        offset_w.reshape(18, Cin, KK).transpose(1, 2, 0)).reshape(Cin, KK * 18)
    mwt = np.ascontiguousarray(
        weight.reshape(Cout, Cin, KK).transpose(1, 2, 0)).reshape(Cin, KK * Cout)
    consts["offwT"] = offwT.astype(ml_dtypes.bfloat16)
    consts["mainWT"] = np.concatenate([mwt, mwt], axis=0).astype(np.float32)
    consts["offb"] = offset_b.reshape(18, 1).astype(np.float32)
    consts["mainb"] = bias.reshape(Cout, 1).astype(np.float32)
    # padded image per batch: [Cin, 133, 133], zeros border (+2 top/left, +3 bot/right)
    xpad = np.zeros((B, Cin, PADW, PADW), np.float32)
    xpad[:, :, 2:2 + H, 2:2 + W] = x
    in_maps = []
    for core in range(8):
        b, half = core // 2, core % 2
        h0 = half * 64
        # slab rows: global-padded rows h0-8 .. h0+75 (84 rows), zero-filled OOB
        slab = np.zeros((Cin, SLABROWS, PADW), np.float32)
        glo = h0 - 8
        lo = max(0, glo)
        hi = min(PADW, glo + SLABROWS)
        slab[:, lo - glo:hi - glo, :] = xpad[b, :, lo:hi, :]
        m = dict(consts)
        flat = slab.reshape(Cin, SLABW)
        # quad slab: word 2j = bf16(s[j]) | bf16(s[j+1])<<16 (x-pair at row y),
        # word 2j+1 = the same pair one slab row (+133) down. One d=2 gather
        # column at pair-index j fetches all four bilinear corners.
        nxt = np.zeros_like(flat)
        nxt[:, :-1] = flat[:, 1:]
        lo16 = flat.astype(ml_dtypes.bfloat16).view(np.uint16).astype(np.uint32)
        hi16 = nxt.astype(ml_dtypes.bfloat16).view(np.uint16).astype(np.uint32)
        p0 = lo16 | (hi16 << 16)
        p1 = np.zeros_like(p0)
        p1[:, :-PADW] = p0[:, PADW:]
        xqq = np.empty((Cin, 2 * SLABW), np.uint32)
        xqq[:, 0::2] = p0
        xqq[:, 1::2] = p1
        m["xpack"] = xqq.view(np.float32)
        m["xbf16"] = flat.astype(ml_dtypes.bfloat16)
        m["cl_lo"] = np.full((128, 576), -2.0 - h0, np.float32)
        m["cl_hi"] = np.full((128, 576), 129.0 - h0, np.float32)
        in_maps.append(m)
    return in_maps


_CACHED = {}

def kernel(x, offset_w, offset_b, weight, bias):
    from concourse.bass_utils import run_bass_kernel_spmd
    x = np.asarray(x, dtype=np.float32)
    offset_w = np.asarray(offset_w, dtype=np.float32)
    offset_b = np.asarray(offset_b, dtype=np.float32)
    weight = np.asarray(weight, dtype=np.float32)
    bias = np.asarray(bias, dtype=np.float32)
    if "nc" not in _CACHED:
        nc = build_program()
        split_waits(nc)
        _CACHED["nc"] = nc
    nc = _CACHED["nc"]
    in_maps = make_in_maps(x, offset_w, offset_b, weight, bias)
    res = run_bass_kernel_spmd(nc, in_maps, list(range(8)))
    out = np.zeros((B, Cout, H, W), dtype=np.float32)
    for core in range(8):
        b, half = core // 2, core % 2
        out[b, :, half * 64:(half + 1) * 64, :] = (
            res.results[core]["out"].reshape(Cout, 64, W))
    return out
